# revision 37
# baseline (speedup 1.0000x reference)
"""Multi-head causal self-attention (B=2, S=2048, D=1024, H=16) on 8 NeuronCores.

Sharding: core c handles batch b = c // 4 and heads 4j..4j+3 where j = c % 4
(tensor-parallel over heads within a 4-core group, data-parallel over batch
across the two groups).  Each core:
  1. loads x[b]^T and its column slices of Wq/Wk/Wv (bf16),
  2. computes Q^T/K^T (feature-major) and V (seq-major) for its 4 heads,
  3. runs causal softmax attention per head entirely on-chip
     (scores are computed transposed, S^T[k, q], so no transposes are needed;
      the softmax denominator comes from an appended ones-column in V),
  4. AllGathers the normalized ctx^T across its 4-core group,
  5. computes its 256-column slice of the output projection (+bias).
The host assembles the 8 disjoint output slices.

Heads are processed in pairs that share the 128 partitions (rows 0-63 = even
head, 64-127 = odd head), so the two S^T matmuls of a pair run in distinct
PE row groups and the exp over both heads is a single fused activation over
a 2-bank PSUM tile.

The whole program is emitted as ONE flat interleaved schedule: per-engine
issue order equals program order, so projection work (K/Q/V) and the output
projection are woven INTO the attention k-loops as per-k-tile filler — the
exp stream (ACT) runs concurrently with projection matmuls and PE never
head-of-line blocks on an exp result.  Pair-1's AllGather is split in three
(qb0-1 / qb2 / qb3) so the output projection for q-tiles 0..11 weaves into
the last attention block, leaving only q-tiles 12..15 after the final
gather.
"""

import math

import ml_dtypes
import numpy as np

import concourse.tile as tile
from concourse import bacc, mybir
from concourse.bass_utils import run_bass_kernel_spmd

BF16_NP = ml_dtypes.bfloat16

B, S, D, H, DH = 2, 2048, 1024, 16, 64
NCORES = 8
GROUP = 4          # cores per batch group
HPC = 4            # heads per core
FPC = HPC * DH     # 256 features per core
QB = 512           # q block width (matmul moving free dim)
KT = 128           # k tile height (partition dim)
SCALE = 1.0 / math.sqrt(S)

F32 = mybir.dt.float32
BF16 = mybir.dt.bfloat16
EXP = mybir.ActivationFunctionType.Exp


def build_program(sim_collective=False, reps=1):
    """sim_collective=True replaces the AllGather with equivalent-volume local
    DMA traffic so the (single-core) TimelineSim cost model can run.
    reps>1 repeats the whole body inside one NEFF (for slope timing)."""
    nc = bacc.Bacc(
        "TRN2",
        target_bir_lowering=False,
        debug=False,
        num_devices=NCORES,
    )

    xT = nc.dram_tensor("xT", [D, S], BF16, kind="ExternalInput").ap()
    wq = nc.dram_tensor("wq", [D, FPC], BF16, kind="ExternalInput").ap()
    wk = nc.dram_tensor("wk", [D, FPC], BF16, kind="ExternalInput").ap()
    wv = nc.dram_tensor("wv", [D, FPC], BF16, kind="ExternalInput").ap()
    wo = nc.dram_tensor("wo", [D, FPC], BF16, kind="ExternalInput").ap()
    bo = nc.dram_tensor("bo", [1, 2 * FPC], F32, kind="ExternalInput").ap()
    tri = nc.dram_tensor("tri", [KT, 2 * KT], BF16, kind="ExternalInput").ap()
    out = nc.dram_tensor("out", [S, FPC], BF16, kind="ExternalOutput").ap()

    with tile.TileContext(nc) as tc:
      for _rep in range(reps):
        with (
            tc.tile_pool(name="cpool", bufs=1) as cpool,
            tc.tile_pool(name="qkvp", bufs=1) as qkvp,
            tc.tile_pool(name="dpool", bufs=1, space="DRAM") as dpool,
            tc.tile_pool(name="stp", bufs=2, space="PSUM") as stp,
            tc.tile_pool(name="ctxp", bufs=2, space="PSUM") as ctxp,
            tc.tile_pool(name="wkp", bufs=1, space="PSUM") as wkp,
            tc.tile_pool(name="attp", bufs=8) as attp,
            tc.tile_pool(name="nrmp", bufs=4) as nrmp,
            tc.tile_pool(name="obp", bufs=2) as obp,
        ):
            # ---- persistent SBUF tensors ---------------------------------
            wq_sb = cpool.tile([128, 8, FPC], BF16)
            wk_sb = cpool.tile([128, 8, FPC], BF16)
            wv_sb = cpool.tile([128, 8, FPC], BF16)
            wo_sb = cpool.tile([128, 8, FPC], BF16)
            xt_sb = cpool.tile([128, 8, S], BF16)
            qT_sb = qkvp.tile([128, 2, S], BF16)   # [dh, head-pair, seq]
            kT_sb = qkvp.tile([128, 2, S], BF16)
            v_sb = qkvp.tile([128, 16, HPC * (DH + 1)], BF16)  # [k, stile, 4*65]
            v4 = v_sb.rearrange("p s (h e) -> p s h e", e=DH + 1)
            ctxg_sb = cpool.tile([128, 8, S], BF16, name="ctxg")

            cc_in0 = dpool.tile([2 * DH, S], BF16)
            cc_in1a = dpool.tile([2 * DH, S // 2], BF16)
            cc_in1b = dpool.tile([2 * DH, S // 4], BF16)
            cc_in1c = dpool.tile([2 * DH, S // 4], BF16)
            cc_out0 = dpool.tile([GROUP * 2 * DH, S], BF16)
            cc_out1a = dpool.tile([GROUP * 2 * DH, S // 2], BF16)
            cc_out1b = dpool.tile([GROUP * 2 * DH, S // 4], BF16)
            cc_out1c = dpool.tile([GROUP * 2 * DH, S // 4], BF16)
            ccg0 = cc_out0.rearrange("(f p) q -> f p q", p=128)
            ccg1a = cc_out1a.rearrange("(f p) q -> f p q", p=128)
            ccg1b = cc_out1b.rearrange("(f p) q -> f p q", p=128)
            ccg1c = cc_out1c.rearrange("(f p) q -> f p q", p=128)

            # ---- exp table preload (runs during the DMA-bound head) ------
            tiny = cpool.tile([1, 16], BF16)
            nc.vector.memset(tiny[:], 0.0)
            tiny2 = cpool.tile([1, 16], F32)
            nc.scalar.activation(tiny2[:], tiny[:], EXP)

            # PE keep-warm: a stream of tiny matmuls during the DMA-bound
            # head so the first real matmuls start at full clock
            warm = wkp.tile([128, QB], F32, tag="pj", bufs=2, name="warm")
            for _ in range(60):
                nc.tensor.matmul(warm[0:16, 0:16], tiny[:], tiny[:],
                                 start=True, stop=True)



            # ---- input DMAs (queue order = need order) -------------------
            xt_dram = xT.rearrange("(t p) m -> t p m", p=128)
            xq_dram = xT.rearrange("(t p) m -> p t m", p=128)

            def dma_w(dst, src, f, t0=0, t1=8):
                fs = slice(f * 128, (f + 1) * 128)
                nc.sync.dma_start(
                    dst[:, t0:t1, fs],
                    src[:, fs].rearrange("(t p) f -> t p f", p=128)
                       [t0:t1].rearrange("t p f -> p t f"),
                )

            def dma_xq(c):
                cs = slice(c * S // 4, (c + 1) * S // 4)
                nc.sync.dma_start(xt_sb[:, :, cs], xq_dram[:, :, cs])

            nc.vector.memset(v4[:, :, :, DH], 1.0)

            def dma_x0(t0):
                nc.sync.dma_start(xt_sb[:, t0:t0 + 2, 0:S // 4],
                                  xq_dram[:, t0:t0 + 2, 0:S // 4])

            dma_w(wk_sb, wk, 0, 0, 4)
            dma_x0(0)
            dma_w(wk_sb, wk, 0, 4, 8)
            dma_x0(2)
            dma_w(wq_sb, wq, 0, 0, 4)
            dma_x0(4)
            dma_w(wq_sb, wq, 0, 4, 8)
            dma_x0(6)
            dma_w(wv_sb, wv, 0)
            dma_w(wv_sb, wv, 1)
            tri_sb = cpool.tile([KT, 2, KT], BF16)
            nc.sync.dma_start(tri_sb[:], tri.rearrange("p (h q) -> p h q", q=KT))
            dma_xq(1)
            dma_xq(2)
            dma_w(wk_sb, wk, 1)
            dma_w(wq_sb, wq, 1)
            dma_xq(3)
            bo_sb = cpool.tile([1, 2 * FPC], F32)
            nc.sync.dma_start(bo_sb[:], bo)
            bias_bc = cpool.tile([128, 2 * FPC], F32)
            nc.gpsimd.partition_broadcast(bias_bc[:], bo_sb[:])
            nc.sync.dma_start(wo_sb[:], wo.rearrange("(t p) f -> p t f", p=128))

            # ---- work-unit emitters --------------------------------------
            def emit_v(s):
                """V projection for seq-tile s (all 4 heads; one stationary
                xt load per t-tile serves the full 256-col moving operand)."""
                ps = wkp.tile([128, QB], F32, tag="pj", bufs=2,
                              name=f"pv_{s}")
                for t in range(8):
                    nc.tensor.matmul(
                        ps[:, 0:4 * DH],
                        xt_sb[:, t, s * 128:(s + 1) * 128],
                        wv_sb[:, t],
                        start=(t == 0),
                        stop=(t == 7),
                    )
                nc.vector.tensor_copy(
                    v4[:, s, :, 0:DH],
                    ps[:, 0:4 * DH].rearrange("p (h e) -> p h e", e=DH),
                )

            def emit_kq(pair, w_sb, dst, qb):
                ps = wkp.tile([128, QB], F32, tag="pj", bufs=2,
                              name=f"pkq_{pair}_{qb}_{0 if w_sb is wk_sb else 1}")
                for t in range(8):
                    nc.tensor.matmul(
                        ps[:],
                        w_sb[:, t, pair * 128:(pair + 1) * 128],
                        xt_sb[:, t, qb * QB:(qb + 1) * QB],
                        start=(t == 0),
                        stop=(t == 7),
                    )
                nc.vector.tensor_copy(
                    dst[:, pair, qb * QB:(qb + 1) * QB], ps[:]
                )

            def emit_o(s):
                """Full output projection for seq-tile s (needs all ctxg).
                Accumulators alternate between the work ring and the score
                banks (free once attention ends) for a deeper pipeline."""
                ps = wkp.tile([128, QB], F32, tag="pj", bufs=2,
                              name=f"ops_{s}")
                for f in range(8):
                    nc.tensor.matmul(
                        ps[:, 0:FPC],
                        ctxg_sb[:, f, s * 128:(s + 1) * 128],
                        wo_sb[:, f],
                        start=(f == 0),
                        stop=(f == 7),
                    )
                ot = obp.tile([128, FPC], BF16, tag="ot", bufs=6,
                              name=f"ot_{s}")
                nc.vector.tensor_add(ot[:], ps[:, 0:FPC], bias_bc[:, 0:FPC])
                nc.gpsimd.dma_start(out[s * 128:(s + 1) * 128, :], ot[:])

            def gather(cin, cout, fhalf, qlo, qhi):
                if sim_collective:
                    for g in range(GROUP):
                        nc.sync.dma_start(
                            cout[g * 2 * DH:(g + 1) * 2 * DH, :], cin[:]
                        )
                else:
                    nc.gpsimd.collective_compute(
                        "AllGather",
                        mybir.AluOpType.bypass,
                        replica_groups=[[0, 1, 2, 3], [4, 5, 6, 7]],
                        ins=[cin.opt()],
                        outs=[cout.opt()],
                    )
                nc.sync.dma_start(
                    ctxg_sb[:, 4 * fhalf:4 * fhalf + 4, qlo:qhi],
                    cout.rearrange("(f p) q -> p f q", p=128),
                )

            # ---- attention block (one (pair, qb), with woven filler) -----
            def att_block(pair, qb, filler, pending=(), copy_ctx=False):
                assert len(filler) <= 4 * (qb + 1)
                h0, h1 = 2 * pair, 2 * pair + 1
                nk = 4 * (qb + 1)
                qs = slice(qb * QB, (qb + 1) * QB)
                ctx0 = ctxp.tile([DH + 1, QB], F32, tag="ctx",
                                 name=f"ctx0_{pair}_{qb}")
                ctx1 = ctxp.tile([DH + 1, QB], F32, tag="ctx",
                                 name=f"ctx1_{pair}_{qb}")

                sts = [None] * nk
                pts = [None] * nk

                def emit_s(ki):
                    ks = slice(ki * KT, (ki + 1) * KT)
                    # columns < off are fully causally masked: never compute
                    off = max(ki * KT - qb * QB, 0)
                    qso = slice(qb * QB + off, (qb + 1) * QB)
                    st = stp.tile([128, 2 * QB], F32, tag="st",
                                  name=f"st_{pair}_{qb}_{ki}")
                    nc.tensor.matmul(
                        st[:, off:QB], kT_sb[0:64, pair, ks],
                        qT_sb[0:64, pair, qso], start=True, stop=True,
                    )
                    nc.tensor.matmul(
                        st[:, QB + off:2 * QB], kT_sb[64:128, pair, ks],
                        qT_sb[64:128, pair, qso], start=True, stop=True,
                    )
                    sts[ki] = st

                def emit_exp(ki):
                    st2 = sts[ki].rearrange("p (h q) -> p h q", q=QB)
                    pt = attp.tile([128, 2, QB], BF16, tag="pt",
                                   name=f"pt_{pair}_{qb}_{ki}")
                    off = ki * KT - qb * QB
                    if off <= 0:
                        nc.scalar.activation(pt[:], st2[:], EXP, scale=SCALE)
                    else:
                        nc.scalar.activation(
                            pt[:, :, off:], st2[:, :, off:], EXP, scale=SCALE,
                        )
                    if off >= 0:
                        nc.vector.tensor_mul(
                            pt[:, :, off:off + KT],
                            pt[:, :, off:off + KT],
                            tri_sb[:],
                        )
                    pts[ki] = pt

                def emit_pv(ki):
                    pt = pts[ki]
                    off = max(ki * KT - qb * QB, 0)
                    nc.tensor.matmul(
                        ctx0[:, off:], v4[:, ki, h0], pt[:, 0, off:],
                        start=(ki == 0), stop=(ki == nk - 1),
                    )
                    nc.tensor.matmul(
                        ctx1[:, off:], v4[:, ki, h1], pt[:, 1, off:],
                        start=(ki == 0), stop=(ki == nk - 1),
                    )

                emit_s(0)
                emit_exp(0)
                if nk > 1:
                    emit_s(1)
                    emit_exp(1)
                for ki in range(nk):
                    if ki + 2 < nk:
                        emit_s(ki + 2)
                        emit_exp(ki + 2)
                    if ki == 2:
                        for p in pending:
                            p()
                    if ki < len(filler):
                        filler[ki]()
                    emit_pv(ki)

                def norm_tail():
                    cnb = nrmp.tile([128, QB], BF16, tag="cn",
                                    name=f"cn_{pair}_{qb}")
                    if copy_ctx:
                        # spill ctx to SBUF right away so the PSUM banks
                        # free before the next block's first PV
                        srcs = []
                        for h, ctx in ((h0, ctx0), (h1, ctx1)):
                            cp = nrmp.tile([DH + 1, QB], F32, tag="cc2",
                                           name=f"cc2_{pair}_{qb}_{h}")
                            nc.vector.tensor_copy(cp[:], ctx[:])
                            srcs.append(cp)
                    else:
                        srcs = [ctx0, ctx1]
                    for h, ctx in ((h0, srcs[0]), (h1, srcs[1])):
                        rc = nrmp.tile([1, QB], F32, tag="rc",
                                       name=f"rc_{pair}_{qb}_{h}")
                        nc.vector.reciprocal(rc[:], ctx[DH:DH + 1, :])
                        bc = nrmp.tile([64, QB], F32, tag="bc",
                                       name=f"bc_{pair}_{qb}_{h}")
                        nc.gpsimd.partition_broadcast(bc[:], rc[:])
                        nc.vector.tensor_mul(
                            cnb[(h % 2) * DH:(h % 2 + 1) * DH, :],
                            ctx[0:DH, :], bc[:],
                        )
                    if pair == 0:
                        nc.sync.dma_start(cc_in0[:, qs], cnb[:])
                    elif qb < 2:
                        nc.sync.dma_start(
                            cc_in1a[:, qb * QB:(qb + 1) * QB], cnb[:])
                    elif qb == 2:
                        nc.sync.dma_start(cc_in1b[:, :], cnb[:])
                    else:
                        nc.sync.dma_start(cc_in1c[:, :], cnb[:])
                return norm_tail

            # ---- the schedule --------------------------------------------
            KQ = lambda pair, w, dst, qb: (lambda: emit_kq(pair, w, dst, qb))
            V = lambda s: (lambda: emit_v(s))

            # prelude: pair-0 qb-0 prerequisites
            emit_kq(0, wk_sb, kT_sb, 0)
            emit_kq(0, wq_sb, qT_sb, 0)
            for s in range(4):
                emit_v(s)

            t00 = att_block(0, 0, [KQ(0, wk_sb, kT_sb, 1),
                                   KQ(0, wq_sb, qT_sb, 1),
                                   V(4), V(5)])
            t01 = att_block(0, 1, [V(6), V(7),
                                   KQ(0, wk_sb, kT_sb, 2),
                                   KQ(0, wq_sb, qT_sb, 2),
                                   V(8), V(9)], pending=[t00])
            t02 = att_block(0, 2, [V(10), V(11),
                                   KQ(0, wk_sb, kT_sb, 3),
                                   KQ(0, wq_sb, qT_sb, 3),
                                   V(12), V(13)], pending=[t01])
            t03 = att_block(0, 3, [V(14), V(15),
                                   KQ(1, wk_sb, kT_sb, 0),
                                   KQ(1, wq_sb, qT_sb, 0)],
                            pending=[t02], copy_ctx=True)
            t03()
            gather(cc_in0, cc_out0, 0, 0, S)

            t10 = att_block(1, 0, [KQ(1, wk_sb, kT_sb, 1),
                                   KQ(1, wq_sb, qT_sb, 1)])
            t11 = att_block(1, 1, [KQ(1, wk_sb, kT_sb, 2),
                                   KQ(1, wq_sb, qT_sb, 2)], pending=[t10],
                            copy_ctx=True)
            t11()
            gather(cc_in1a, cc_out1a, 1, 0, S // 2)
            t12 = att_block(1, 2, [KQ(1, wk_sb, kT_sb, 3),
                                   KQ(1, wq_sb, qT_sb, 3)], copy_ctx=True)
            t12()
            gather(cc_in1b, cc_out1b, 1, S // 2, 3 * S // 4)
            t13 = att_block(1, 3, [])
            # first two O units before the final norm: their bias-adds land
            # ahead of the norm chain on DVE, keeping the O ring turning
            emit_o(0)
            emit_o(1)
            t13()
            gather(cc_in1c, cc_out1c, 1, 3 * S // 4, S)
            for s in range(2, 12):
                emit_o(s)
            # tail q-tiles: f0-3 accumulate while the last gather flies,
            # f4-7 + bias + store once it lands
            # accumulators live in the (now free) score banks — one s-tile
            # per PSUM bank, since a group start clears its whole bank
            pstl = [stp.tile([128, 2 * QB], F32, tag="st",
                             name=f"opst_{i}") for i in range(2)]
            for i, ps in enumerate(pstl):
                for j in (0, 1):
                    s = 12 + 2 * i + j
                    for f in range(4):
                        nc.tensor.matmul(
                            ps[:, j * QB:j * QB + FPC],
                            ctxg_sb[:, f, s * 128:(s + 1) * 128],
                            wo_sb[:, f],
                            start=(f == 0),
                            stop=False,
                        )
            warm2 = wkp.tile([128, QB], F32, tag="pj", bufs=2, name="warm2")
            for _ in range(17):
                nc.tensor.matmul(warm2[0:16, :], xt_sb[0:1, 0, 0:16],
                                 xt_sb[0:1, 0, 0:QB], start=True, stop=True)
            for i, ps in enumerate(pstl):
                for j in (0, 1):
                    s = 12 + 2 * i + j
                    for f in range(4, 8):
                        nc.tensor.matmul(
                            ps[:, j * QB:j * QB + FPC],
                            ctxg_sb[:, f, s * 128:(s + 1) * 128],
                            wo_sb[:, f],
                            start=False,
                            stop=(f == 7),
                        )
                ot = obp.tile([128, 2, FPC], BF16, tag="ot2", bufs=2,
                              name=f"ott_{i}")
                psv = ps.rearrange("p (j q) -> p j q", q=QB)
                nc.vector.tensor_add(
                    ot[:], psv[:, :, 0:FPC],
                    bias_bc.rearrange("p (s f) -> p s f", f=FPC))
                s0 = 12 + 2 * i
                dst = out[s0 * 128:(s0 + 2) * 128, :].rearrange(
                    "(s p) f -> p s f", p=128)
                if i == 0:
                    nc.gpsimd.dma_start(dst, ot[:])
                else:
                    nc.sync.dma_start(dst, ot[:])

    nc.compile()
    return nc


_PROGRAM = None


def _get_program():
    global _PROGRAM
    if _PROGRAM is None:
        _PROGRAM = build_program()
    return _PROGRAM


def _make_tri():
    # tri[i, j] = 1 where key-offset i <= query-offset j (allowed); the two
    # copies along the free dim serve the two heads of a fused pair tile
    i = np.arange(KT)[:, None]
    j = np.arange(KT)[None, :]
    t = (i <= j).astype(BF16_NP)
    return np.concatenate([t, t], axis=1)


def make_in_maps(x, Wq, Wk, Wv, Wo, bo):
    tri_arr = _make_tri()
    xTs = [np.ascontiguousarray(x[b].T.astype(BF16_NP)) for b in range(B)]
    # Wo rows permuted to match the gathered ctx^T feature order:
    # gather0 rows = (rank j, heads 4j+0, 4j+1), gather1 = (rank j, 4j+2, 4j+3)
    perm = [4 * j + p for g in range(2) for j in range(GROUP)
            for p in (2 * g, 2 * g + 1)]
    Wo_perm = Wo.reshape(H, DH, D)[perm].reshape(D, D)
    in_maps = []
    for c in range(NCORES):
        b, j = divmod(c, GROUP)
        cols = slice(FPC * j, FPC * (j + 1))
        in_maps.append({
            "xT": xTs[b],
            "wq": np.ascontiguousarray(Wq[:, cols].astype(BF16_NP)),
            "wk": np.ascontiguousarray(Wk[:, cols].astype(BF16_NP)),
            "wv": np.ascontiguousarray(Wv[:, cols].astype(BF16_NP)),
            "wo": np.ascontiguousarray(Wo_perm[:, cols].astype(BF16_NP)),
            "bo": np.ascontiguousarray(
                np.concatenate([bo[cols], bo[cols]])[None, :]),
            "tri": tri_arr,
        })
    return in_maps


def kernel(x, Wq, Wk, Wv, Wo, bo):
    x = np.ascontiguousarray(np.asarray(x, np.float32))
    Wq = np.asarray(Wq, np.float32)
    Wk = np.asarray(Wk, np.float32)
    Wv = np.asarray(Wv, np.float32)
    Wo = np.asarray(Wo, np.float32)
    bo = np.asarray(bo, np.float32)

    in_maps = make_in_maps(x, Wq, Wk, Wv, Wo, bo)
    nc = _get_program()
    results = run_bass_kernel_spmd(nc, in_maps, list(range(NCORES))).results

    out = np.empty((B, S, D), np.float32)
    for c in range(NCORES):
        b, j = divmod(c, GROUP)
        out[b, :, FPC * j:FPC * (j + 1)] = results[c]["out"].astype(
            np.float32)
    return out


# revision 38
# speedup vs baseline: 5.7404x; 5.7404x over previous
"""Multi-head causal self-attention (B=2, S=2048, D=1024, H=16) on 8 NeuronCores.

Sharding: core c handles batch b = c // 4 and heads 4j..4j+3 where j = c % 4
(tensor-parallel over heads within a 4-core group, data-parallel over batch
across the two groups).  Each core:
  1. loads x[b]^T and its column slices of Wq/Wk/Wv (bf16),
  2. computes Q^T/K^T (feature-major) and V (seq-major) for its 4 heads,
  3. runs causal softmax attention per head entirely on-chip
     (scores are computed transposed, S^T[k, q], so no transposes are needed;
      the softmax denominator comes from an appended ones-column in V),
  4. AllGathers the normalized ctx^T across its 4-core group,
  5. computes its 256-column slice of the output projection (+bias).
The host assembles the 8 disjoint output slices.

Heads are processed in pairs that share the 128 partitions (rows 0-63 = even
head, 64-127 = odd head), so the two S^T matmuls of a pair run in distinct
PE row groups and the exp over both heads is a single fused activation over
a 2-bank PSUM tile.

The whole program is emitted as ONE flat interleaved schedule: per-engine
issue order equals program order, so projection work (K/Q/V) and the output
projection are woven INTO the attention k-loops as per-k-tile filler — the
exp stream (ACT) runs concurrently with projection matmuls and PE never
head-of-line blocks on an exp result.  Pair-1's AllGather is split in three
(qb0-1 / qb2 / qb3) so the output projection for q-tiles 0..11 weaves into
the last attention block, leaving only q-tiles 12..15 after the final
gather.
"""

import math

import ml_dtypes
import numpy as np

import concourse.tile as tile
from concourse import bacc, mybir
from concourse.bass_utils import run_bass_kernel_spmd

BF16_NP = ml_dtypes.bfloat16

B, S, D, H, DH = 2, 2048, 1024, 16, 64
NCORES = 8
GROUP = 4          # cores per batch group
HPC = 4            # heads per core
FPC = HPC * DH     # 256 features per core
QB = 512           # q block width (matmul moving free dim)
KT = 128           # k tile height (partition dim)
SCALE = 1.0 / math.sqrt(S)

F32 = mybir.dt.float32
BF16 = mybir.dt.bfloat16
EXP = mybir.ActivationFunctionType.Exp


def build_program(sim_collective=False, reps=1):
    """sim_collective=True replaces the AllGather with equivalent-volume local
    DMA traffic so the (single-core) TimelineSim cost model can run.
    reps>1 repeats the whole body inside one NEFF (for slope timing)."""
    nc = bacc.Bacc(
        "TRN2",
        target_bir_lowering=False,
        debug=False,
        num_devices=NCORES,
    )

    xT = nc.dram_tensor("xT", [D, S], BF16, kind="ExternalInput").ap()
    wq = nc.dram_tensor("wq", [D, FPC], BF16, kind="ExternalInput").ap()
    wk = nc.dram_tensor("wk", [D, FPC], BF16, kind="ExternalInput").ap()
    wv = nc.dram_tensor("wv", [D, FPC], BF16, kind="ExternalInput").ap()
    wo = nc.dram_tensor("wo", [D, FPC], BF16, kind="ExternalInput").ap()
    bo = nc.dram_tensor("bo", [1, 2 * FPC], F32, kind="ExternalInput").ap()
    tri = nc.dram_tensor("tri", [KT, 2 * KT], BF16, kind="ExternalInput").ap()
    out = nc.dram_tensor("out", [S, FPC], BF16, kind="ExternalOutput").ap()

    with tile.TileContext(nc) as tc:
      for _rep in range(reps):
        with (
            tc.tile_pool(name="cpool", bufs=1) as cpool,
            tc.tile_pool(name="qkvp", bufs=1) as qkvp,
            tc.tile_pool(name="dpool", bufs=1, space="DRAM") as dpool,
            tc.tile_pool(name="stp", bufs=2, space="PSUM") as stp,
            tc.tile_pool(name="ctxp", bufs=2, space="PSUM") as ctxp,
            tc.tile_pool(name="wkp", bufs=1, space="PSUM") as wkp,
            tc.tile_pool(name="attp", bufs=8) as attp,
            tc.tile_pool(name="nrmp", bufs=4) as nrmp,
            tc.tile_pool(name="obp", bufs=2) as obp,
        ):
            # ---- persistent SBUF tensors ---------------------------------
            wq_sb = cpool.tile([128, 8, FPC], BF16)
            wk_sb = cpool.tile([128, 8, FPC], BF16)
            wv_sb = cpool.tile([128, 8, FPC], BF16)
            wo_sb = cpool.tile([128, 8, FPC], BF16)
            xt_sb = cpool.tile([128, 8, S], BF16)
            qT_sb = qkvp.tile([128, 2, S], BF16)   # [dh, head-pair, seq]
            kT_sb = qkvp.tile([128, 2, S], BF16)
            v_sb = qkvp.tile([128, 16, HPC * (DH + 1)], BF16)  # [k, stile, 4*65]
            v4 = v_sb.rearrange("p s (h e) -> p s h e", e=DH + 1)
            ctxg_sb = cpool.tile([128, 8, S], BF16, name="ctxg")

            cc_in0 = dpool.tile([2 * DH, S], BF16)
            cc_in1a = dpool.tile([2 * DH, S // 2], BF16)
            cc_in1b = dpool.tile([2 * DH, S // 4], BF16)
            cc_in1c = dpool.tile([2 * DH, S // 4], BF16)
            cc_out0 = dpool.tile([GROUP * 2 * DH, S], BF16)
            cc_out1a = dpool.tile([GROUP * 2 * DH, S // 2], BF16)
            cc_out1b = dpool.tile([GROUP * 2 * DH, S // 4], BF16)
            cc_out1c = dpool.tile([GROUP * 2 * DH, S // 4], BF16)
            ccg0 = cc_out0.rearrange("(f p) q -> f p q", p=128)
            ccg1a = cc_out1a.rearrange("(f p) q -> f p q", p=128)
            ccg1b = cc_out1b.rearrange("(f p) q -> f p q", p=128)
            ccg1c = cc_out1c.rearrange("(f p) q -> f p q", p=128)

            # ---- exp table preload (runs during the DMA-bound head) ------
            tiny = cpool.tile([1, 16], BF16)
            nc.vector.memset(tiny[:], 0.0)
            tiny2 = cpool.tile([1, 16], F32)
            nc.scalar.activation(tiny2[:], tiny[:], EXP)

            # PE keep-warm: a stream of tiny matmuls during the DMA-bound
            # head so the first real matmuls start at full clock
            warm = wkp.tile([128, QB], F32, tag="pj", bufs=2, name="warm")
            for _ in range(60):
                nc.tensor.matmul(warm[0:16, 0:16], tiny[:], tiny[:],
                                 start=True, stop=True)



            # ---- input DMAs (queue order = need order) -------------------
            xt_dram = xT.rearrange("(t p) m -> t p m", p=128)
            xq_dram = xT.rearrange("(t p) m -> p t m", p=128)

            def dma_w(dst, src, f, t0=0, t1=8):
                fs = slice(f * 128, (f + 1) * 128)
                nc.sync.dma_start(
                    dst[:, t0:t1, fs],
                    src[:, fs].rearrange("(t p) f -> t p f", p=128)
                       [t0:t1].rearrange("t p f -> p t f"),
                )

            def dma_xq(c):
                cs = slice(c * S // 4, (c + 1) * S // 4)
                nc.sync.dma_start(xt_sb[:, :, cs], xq_dram[:, :, cs])

            nc.vector.memset(v4[:, :, :, DH], 1.0)

            def dma_x0(t0):
                nc.sync.dma_start(xt_sb[:, t0:t0 + 2, 0:S // 4],
                                  xq_dram[:, t0:t0 + 2, 0:S // 4])

            dma_w(wk_sb, wk, 0, 0, 4)
            dma_x0(0)
            dma_w(wk_sb, wk, 0, 4, 8)
            dma_x0(2)
            dma_w(wq_sb, wq, 0, 0, 4)
            dma_x0(4)
            dma_w(wq_sb, wq, 0, 4, 8)
            dma_x0(6)
            dma_w(wv_sb, wv, 0)
            dma_w(wv_sb, wv, 1)
            tri_sb = cpool.tile([KT, 2, KT], BF16)
            nc.sync.dma_start(tri_sb[:], tri.rearrange("p (h q) -> p h q", q=KT))
            dma_xq(1)
            dma_xq(2)
            dma_w(wk_sb, wk, 1)
            dma_w(wq_sb, wq, 1)
            dma_xq(3)
            bo_sb = cpool.tile([1, 2 * FPC], F32)
            nc.sync.dma_start(bo_sb[:], bo)
            bias_bc = cpool.tile([128, 2 * FPC], F32)
            nc.gpsimd.partition_broadcast(bias_bc[:], bo_sb[:])
            nc.sync.dma_start(wo_sb[:], wo.rearrange("(t p) f -> p t f", p=128))

            # ---- work-unit emitters --------------------------------------
            def emit_v(s):
                """V projection for seq-tile s (all 4 heads; one stationary
                xt load per t-tile serves the full 256-col moving operand)."""
                ps = wkp.tile([128, QB], F32, tag="pj", bufs=2,
                              name=f"pv_{s}")
                for t in range(8):
                    nc.tensor.matmul(
                        ps[:, 0:4 * DH],
                        xt_sb[:, t, s * 128:(s + 1) * 128],
                        wv_sb[:, t],
                        start=(t == 0),
                        stop=(t == 7),
                    )
                nc.vector.tensor_copy(
                    v4[:, s, :, 0:DH],
                    ps[:, 0:4 * DH].rearrange("p (h e) -> p h e", e=DH),
                )

            def emit_kq(pair, w_sb, dst, qb):
                ps = wkp.tile([128, QB], F32, tag="pj", bufs=2,
                              name=f"pkq_{pair}_{qb}_{0 if w_sb is wk_sb else 1}")
                for t in range(8):
                    nc.tensor.matmul(
                        ps[:],
                        w_sb[:, t, pair * 128:(pair + 1) * 128],
                        xt_sb[:, t, qb * QB:(qb + 1) * QB],
                        start=(t == 0),
                        stop=(t == 7),
                    )
                nc.vector.tensor_copy(
                    dst[:, pair, qb * QB:(qb + 1) * QB], ps[:]
                )

            def emit_o(s):
                """Full output projection for seq-tile s (needs all ctxg).
                Accumulators alternate between the work ring and the score
                banks (free once attention ends) for a deeper pipeline."""
                ps = wkp.tile([128, QB], F32, tag="pj", bufs=2,
                              name=f"ops_{s}")
                for f in range(8):
                    nc.tensor.matmul(
                        ps[:, 0:FPC],
                        ctxg_sb[:, f, s * 128:(s + 1) * 128],
                        wo_sb[:, f],
                        start=(f == 0),
                        stop=(f == 7),
                    )
                ot = obp.tile([128, FPC], BF16, tag="ot", bufs=6,
                              name=f"ot_{s}")
                nc.vector.tensor_add(ot[:], ps[:, 0:FPC], bias_bc[:, 0:FPC])
                nc.gpsimd.dma_start(out[s * 128:(s + 1) * 128, :], ot[:])

            def gather(cin, cout, fhalf, qlo, qhi):
                if sim_collective:
                    for g in range(GROUP):
                        nc.sync.dma_start(
                            cout[g * 2 * DH:(g + 1) * 2 * DH, :], cin[:]
                        )
                else:
                    nc.gpsimd.collective_compute(
                        "AllGather",
                        mybir.AluOpType.bypass,
                        replica_groups=[[0, 1, 2, 3], [4, 5, 6, 7]],
                        ins=[cin.opt()],
                        outs=[cout.opt()],
                    )
                nc.sync.dma_start(
                    ctxg_sb[:, 4 * fhalf:4 * fhalf + 4, qlo:qhi],
                    cout.rearrange("(f p) q -> p f q", p=128),
                )

            # ---- attention block (one (pair, qb), with woven filler) -----
            def att_block(pair, qb, filler, pending=(), copy_ctx=False):
                assert len(filler) <= 4 * (qb + 1)
                h0, h1 = 2 * pair, 2 * pair + 1
                nk = 4 * (qb + 1)
                qs = slice(qb * QB, (qb + 1) * QB)
                ctx0 = ctxp.tile([DH + 1, QB], F32, tag="ctx",
                                 name=f"ctx0_{pair}_{qb}")
                ctx1 = ctxp.tile([DH + 1, QB], F32, tag="ctx",
                                 name=f"ctx1_{pair}_{qb}")

                sts = [None] * nk
                pts = [None] * nk

                def emit_s(ki):
                    ks = slice(ki * KT, (ki + 1) * KT)
                    # columns < off are fully causally masked: never compute
                    off = max(ki * KT - qb * QB, 0)
                    qso = slice(qb * QB + off, (qb + 1) * QB)
                    st = stp.tile([128, 2 * QB], F32, tag="st",
                                  name=f"st_{pair}_{qb}_{ki}")
                    nc.tensor.matmul(
                        st[:, off:QB], kT_sb[0:64, pair, ks],
                        qT_sb[0:64, pair, qso], start=True, stop=True,
                    )
                    nc.tensor.matmul(
                        st[:, QB + off:2 * QB], kT_sb[64:128, pair, ks],
                        qT_sb[64:128, pair, qso], start=True, stop=True,
                    )
                    sts[ki] = st

                def emit_exp(ki):
                    st2 = sts[ki].rearrange("p (h q) -> p h q", q=QB)
                    pt = attp.tile([128, 2, QB], BF16, tag="pt",
                                   name=f"pt_{pair}_{qb}_{ki}")
                    off = ki * KT - qb * QB
                    if off <= 0:
                        nc.scalar.activation(pt[:], st2[:], EXP, scale=SCALE)
                    else:
                        nc.scalar.activation(
                            pt[:, :, off:], st2[:, :, off:], EXP, scale=SCALE,
                        )
                    if off >= 0:
                        nc.vector.tensor_mul(
                            pt[:, :, off:off + KT],
                            pt[:, :, off:off + KT],
                            tri_sb[:],
                        )
                    pts[ki] = pt

                def emit_pv(ki):
                    pt = pts[ki]
                    off = max(ki * KT - qb * QB, 0)
                    nc.tensor.matmul(
                        ctx0[:, off:], v4[:, ki, h0], pt[:, 0, off:],
                        start=(ki == 0), stop=(ki == nk - 1),
                    )
                    nc.tensor.matmul(
                        ctx1[:, off:], v4[:, ki, h1], pt[:, 1, off:],
                        start=(ki == 0), stop=(ki == nk - 1),
                    )

                emit_s(0)
                emit_exp(0)
                if nk > 1:
                    emit_s(1)
                    emit_exp(1)
                for ki in range(nk):
                    if ki < len(filler):
                        filler[ki]()
                    if ki + 2 < nk:
                        emit_s(ki + 2)
                        emit_exp(ki + 2)
                    if ki == 2:
                        for p in pending:
                            p()
                    emit_pv(ki)

                def norm_tail():
                    cnb = nrmp.tile([128, QB], BF16, tag="cn",
                                    name=f"cn_{pair}_{qb}")
                    if copy_ctx:
                        # spill ctx to SBUF right away so the PSUM banks
                        # free before the next block's first PV
                        srcs = []
                        for h, ctx in ((h0, ctx0), (h1, ctx1)):
                            cp = nrmp.tile([DH + 1, QB], F32, tag="cc2",
                                           name=f"cc2_{pair}_{qb}_{h}")
                            nc.vector.tensor_copy(cp[:], ctx[:])
                            srcs.append(cp)
                    else:
                        srcs = [ctx0, ctx1]
                    for h, ctx in ((h0, srcs[0]), (h1, srcs[1])):
                        rc = nrmp.tile([1, QB], F32, tag="rc",
                                       name=f"rc_{pair}_{qb}_{h}")
                        nc.vector.reciprocal(rc[:], ctx[DH:DH + 1, :])
                        bc = nrmp.tile([64, QB], F32, tag="bc",
                                       name=f"bc_{pair}_{qb}_{h}")
                        nc.gpsimd.partition_broadcast(bc[:], rc[:])
                        nc.vector.tensor_mul(
                            cnb[(h % 2) * DH:(h % 2 + 1) * DH, :],
                            ctx[0:DH, :], bc[:],
                        )
                    if pair == 0:
                        nc.sync.dma_start(cc_in0[:, qs], cnb[:])
                    elif qb < 2:
                        nc.sync.dma_start(
                            cc_in1a[:, qb * QB:(qb + 1) * QB], cnb[:])
                    elif qb == 2:
                        nc.sync.dma_start(cc_in1b[:, :], cnb[:])
                    else:
                        nc.sync.dma_start(cc_in1c[:, :], cnb[:])
                return norm_tail

            # ---- the schedule --------------------------------------------
            KQ = lambda pair, w, dst, qb: (lambda: emit_kq(pair, w, dst, qb))
            V = lambda s: (lambda: emit_v(s))

            # prelude: pair-0 qb-0 prerequisites
            emit_kq(0, wk_sb, kT_sb, 0)
            emit_kq(0, wq_sb, qT_sb, 0)
            for s in range(4):
                emit_v(s)

            t00 = att_block(0, 0, [KQ(0, wk_sb, kT_sb, 1),
                                   KQ(0, wq_sb, qT_sb, 1),
                                   V(4), V(5)])
            t01 = att_block(0, 1, [V(6), V(7),
                                   KQ(0, wk_sb, kT_sb, 2),
                                   KQ(0, wq_sb, qT_sb, 2),
                                   V(8), V(9)], pending=[t00])
            t02 = att_block(0, 2, [V(10), V(11),
                                   KQ(0, wk_sb, kT_sb, 3),
                                   KQ(0, wq_sb, qT_sb, 3),
                                   V(12), V(13)], pending=[t01])
            t03 = att_block(0, 3, [V(14), V(15),
                                   KQ(1, wk_sb, kT_sb, 0),
                                   KQ(1, wq_sb, qT_sb, 0)],
                            pending=[t02], copy_ctx=True)
            t03()
            gather(cc_in0, cc_out0, 0, 0, S)

            t10 = att_block(1, 0, [KQ(1, wk_sb, kT_sb, 1),
                                   KQ(1, wq_sb, qT_sb, 1)])
            t11 = att_block(1, 1, [KQ(1, wk_sb, kT_sb, 2),
                                   KQ(1, wq_sb, qT_sb, 2)], pending=[t10],
                            copy_ctx=True)
            t11()
            gather(cc_in1a, cc_out1a, 1, 0, S // 2)
            t12 = att_block(1, 2, [KQ(1, wk_sb, kT_sb, 3),
                                   KQ(1, wq_sb, qT_sb, 3)], copy_ctx=True)
            t12()
            gather(cc_in1b, cc_out1b, 1, S // 2, 3 * S // 4)
            t13 = att_block(1, 3, [])
            # first two O units before the final norm: their bias-adds land
            # ahead of the norm chain on DVE, keeping the O ring turning
            emit_o(0)
            emit_o(1)
            t13()
            gather(cc_in1c, cc_out1c, 1, 3 * S // 4, S)
            for s in range(2, 12):
                emit_o(s)
            # tail q-tiles: f0-3 accumulate while the last gather flies,
            # f4-7 + bias + store once it lands
            # accumulators live in the (now free) score banks — one s-tile
            # per PSUM bank, since a group start clears its whole bank
            pstl = [stp.tile([128, 2 * QB], F32, tag="st",
                             name=f"opst_{i}") for i in range(2)]
            for i, ps in enumerate(pstl):
                for j in (0, 1):
                    s = 12 + 2 * i + j
                    for f in range(4):
                        nc.tensor.matmul(
                            ps[:, j * QB:j * QB + FPC],
                            ctxg_sb[:, f, s * 128:(s + 1) * 128],
                            wo_sb[:, f],
                            start=(f == 0),
                            stop=False,
                        )
            warm2 = wkp.tile([128, QB], F32, tag="pj", bufs=2, name="warm2")
            for _ in range(21):
                nc.tensor.matmul(warm2[0:16, :], xt_sb[0:1, 0, 0:16],
                                 xt_sb[0:1, 0, 0:QB], start=True, stop=True)
            for i, ps in enumerate(pstl):
                for j in (0, 1):
                    s = 12 + 2 * i + j
                    for f in range(4, 8):
                        nc.tensor.matmul(
                            ps[:, j * QB:j * QB + FPC],
                            ctxg_sb[:, f, s * 128:(s + 1) * 128],
                            wo_sb[:, f],
                            start=False,
                            stop=(f == 7),
                        )
                ot = obp.tile([128, 2, FPC], BF16, tag="ot2", bufs=2,
                              name=f"ott_{i}")
                psv = ps.rearrange("p (j q) -> p j q", q=QB)
                nc.vector.tensor_add(
                    ot[:], psv[:, :, 0:FPC],
                    bias_bc.rearrange("p (s f) -> p s f", f=FPC))
                s0 = 12 + 2 * i
                dst = out[s0 * 128:(s0 + 2) * 128, :].rearrange(
                    "(s p) f -> p s f", p=128)
                if i == 0:
                    nc.gpsimd.dma_start(dst, ot[:])
                else:
                    nc.sync.dma_start(dst, ot[:])

    nc.compile()
    return nc


_PROGRAM = None


def _get_program():
    global _PROGRAM
    if _PROGRAM is None:
        _PROGRAM = build_program()
    return _PROGRAM


def _make_tri():
    # tri[i, j] = 1 where key-offset i <= query-offset j (allowed); the two
    # copies along the free dim serve the two heads of a fused pair tile
    i = np.arange(KT)[:, None]
    j = np.arange(KT)[None, :]
    t = (i <= j).astype(BF16_NP)
    return np.concatenate([t, t], axis=1)


def make_in_maps(x, Wq, Wk, Wv, Wo, bo):
    tri_arr = _make_tri()
    xTs = [np.ascontiguousarray(x[b].T.astype(BF16_NP)) for b in range(B)]
    # Wo rows permuted to match the gathered ctx^T feature order:
    # gather0 rows = (rank j, heads 4j+0, 4j+1), gather1 = (rank j, 4j+2, 4j+3)
    perm = [4 * j + p for g in range(2) for j in range(GROUP)
            for p in (2 * g, 2 * g + 1)]
    Wo_perm = Wo.reshape(H, DH, D)[perm].reshape(D, D)
    in_maps = []
    for c in range(NCORES):
        b, j = divmod(c, GROUP)
        cols = slice(FPC * j, FPC * (j + 1))
        in_maps.append({
            "xT": xTs[b],
            "wq": np.ascontiguousarray(Wq[:, cols].astype(BF16_NP)),
            "wk": np.ascontiguousarray(Wk[:, cols].astype(BF16_NP)),
            "wv": np.ascontiguousarray(Wv[:, cols].astype(BF16_NP)),
            "wo": np.ascontiguousarray(Wo_perm[:, cols].astype(BF16_NP)),
            "bo": np.ascontiguousarray(
                np.concatenate([bo[cols], bo[cols]])[None, :]),
            "tri": tri_arr,
        })
    return in_maps


def kernel(x, Wq, Wk, Wv, Wo, bo):
    x = np.ascontiguousarray(np.asarray(x, np.float32))
    Wq = np.asarray(Wq, np.float32)
    Wk = np.asarray(Wk, np.float32)
    Wv = np.asarray(Wv, np.float32)
    Wo = np.asarray(Wo, np.float32)
    bo = np.asarray(bo, np.float32)

    in_maps = make_in_maps(x, Wq, Wk, Wv, Wo, bo)
    nc = _get_program()
    results = run_bass_kernel_spmd(nc, in_maps, list(range(NCORES))).results

    out = np.empty((B, S, D), np.float32)
    for c in range(NCORES):
        b, j = divmod(c, GROUP)
        out[b, :, FPC * j:FPC * (j + 1)] = results[c]["out"].astype(
            np.float32)
    return out
